# revision 1
# baseline (speedup 1.0000x reference)
"""Soft-KNN NLL loss (ASKLoss) Trainium2 kernel.

Problem: x[1024,128] queries vs x_ref[50000,128] bank,
  score = -||x - xr||_2, probs = softmax over the 50000 refs,
  soft_nns = probs @ onehot(y_ref) + 1e-6, loss = -mean(log(soft_nns[b, y[b]])).

Strategy: data-parallel over the query batch across the 8 cores (128
queries/core, full 50000-ref bank streamed through each core in fp16).

Per core:
  - d2[b, n] = ||x_b||^2 + ||xr_n||^2 - 2<x_b, xr_n> built as:
      PE:  psum  = (-2 x^T)^T @ xr^T        (K=128 fp16 matmul)
      PE:  psum += ones^T @ xrnorm          (K=1 fp16 matmul, rank-1 row add)
  - sqrt is split across TWO engines (ACT is otherwise the hard bottleneck —
    it is the only table-based sqrt/exp engine at 1 elem/lane/cycle):
      classes 0..K_CUT-1: ACT Sqrt(psum + ||x_b||^2) -> s fp16
      classes K_CUT..9:   DVE custom ops (quadratic minimax rsqrt seed +
                          one Newton step, then *u) -> s fp16.  Valid because
                          d2 lies in a ~3-octave range; pad slots get
                          xrnorm=380 so they stay in range and exp(-s) ~ 0.
  - refs are class-sorted host-side into per-class slots, so one ACT op per
    class: Exp(-s[slot]) with accum_out -> per-class sum [128, 10].
    ACT-computed classes are exp'd FIRST, overlapping the DVE sqrt work for
    the later classes (cross-phase pipelining).
  - The Sqrt/Exp ACT table sets are distinct; a Copy-op fence on ACT (zeros
    tile from the last ACT Sqrt) keeps the scheduler from interleaving them.

Host: concat the per-core [128, 10] class sums, compute the NLL in f64.
"""

import os
import re

import numpy as np

import concourse.bass as bass
import concourse.dve_ops as dops
import concourse.mybir as mybir
import concourse.tile as tile
from concourse import bacc
from concourse.bass_utils import run_bass_kernel_spmd
from concourse.dve_spec import C0, C1, C2, Spec, Src0, Src1

B, N, D, C = 1024, 50000, 128, 10
N_CORES = 8
B_LOC = B // N_CORES           # 128 queries per core: one partition block

PAD_NORM = 380.0               # pad slots: in seed range, exp(-sqrt(~510)) ~ 0
GROUP = 1024                   # ref columns per PSUM tile (2 banks; 4 tiles live)
K_CUT = 5 

# quadratic minimax rsqrt seed over u in [95, 580]; 1 Newton -> s rel err
# <= 0.4% on the data range (d2 in [100, 455])
SEED_CONSTS = (0.12698873227399485, -0.00033429848826787336, 3.39174306115537e-07)

F16 = mybir.dt.float16
F32 = mybir.dt.float32

LAST = {}                      # test harness introspection
_MODULE_CACHE = {}             # caps tuple -> (nc, names); reuse across calls

# ---- custom DVE ops: rsqrt seed + fused Newton*u ---------------------------


def _ref_seed(in0, in1, c0, c1, c2):
    return c0 + in0 * (c1 + in0 * c2)


def _ref_nr(in0, in1, c0, c1, c2):
    u2 = in0 + c0
    return u2 * (in1 * (c1 - c2 * (u2 * (in1 * in1))))


def _register_op(name, body, ref):
    if name in dops._SUB_OPCODE_FOR_NAME:
        for op in dops.OPS:
            if op.name == name:
                return op
    probe = dops.DveOp(name, Spec(body=body, reference=ref), subdim=False,
                       uops_sha={})
    dops.OPS.append(probe)
    dops._SUB_OPCODE_FOR_NAME[name] = (
        dops._CUSTOM_DVE_ROW_BASE + len(dops.OPS) - 1
    )
    assert dops._SUB_OPCODE_FOR_NAME[name] < 0x20
    shas = {}
    for ver in ("v3", "v4"):
        try:
            probe.compile(ver)
            shas[ver] = probe.uops_sha.get(ver)
        except ValueError as e:
            shas[ver] = re.search(r'="([0-9a-f]+)"', str(e)).group(1)
    final = dops.DveOp(name, Spec(body=body, reference=ref), subdim=False,
                       uops_sha=shas)
    dops.OPS[-1] = final
    dops.CUSTOM_DVE_SPECS[name] = final.spec
    return final


RSQRT_SEED_ANT = _register_op(
    "RSQRT_SEED_ANT", C0 + Src0 * (C1 + Src0 * C2), _ref_seed
)
_U2 = Src0 + C0            # u + ||x_b||^2 (C0 = per-partition xnorm AP)
SQRT_NR_BIAS_ANT = _register_op(
    "SQRT_NR_BIAS_ANT",
    _U2 * (Src1 * (C1 - C2 * (_U2 * (Src1 * Src1)))),
    _ref_nr,
)


def _build_module(caps):
    """Build the SPMD Bass module for per-class slot sizes `caps` (len C)."""
    caps = [int(c) for c in caps]
    offs = np.concatenate([[0], np.cumsum(caps)]).astype(int)
    n_pad = int(offs[-1])
    max_cap = max(caps)
    CUT_ADJ = 0
    cut = int(offs[K_CUT]) + CUT_ADJ   # chain boundary (may sit inside class K_CUT's slot)

    nc = bacc.Bacc(
        "TRN2",
        target_bir_lowering=False,
        debug=False,
        enable_asserts=True,
        num_devices=N_CORES,
    )

    xT2_d = nc.dram_tensor("xT2", [D, B_LOC], F16, kind="ExternalInput")
    xrT_d = nc.dram_tensor("xrT", [D, n_pad], F16, kind="ExternalInput")
    xrn_d = nc.dram_tensor("xrnorm", [1, n_pad], F16, kind="ExternalInput")
    xn_d = nc.dram_tensor("xnorm", [128, 1], F32, kind="ExternalInput")
    sc0_d = nc.dram_tensor("seedc0", [128, 1], F32, kind="ExternalInput")
    sc1_d = nc.dram_tensor("seedc1", [128, 1], F32, kind="ExternalInput")
    cls_d = nc.dram_tensor("cls", [128, C], F32, kind="ExternalOutput")

    def chain_groups(a, b):
        bounds = list(range(a, b, GROUP)) + [b]
        return list(zip(bounds[:-1], bounds[1:]))

    act_groups = chain_groups(0, cut)
    dve_groups = chain_groups(cut, n_pad)
    # interleave GROUPS by cumulative CONSUMPTION TIME (ACT ~1.0us per
    # 1024-group, DVE ~2.4us): a count-proportional or pair-blocked order
    # head-of-line-blocks the in-order PE on whichever consumer lags.
    # DMA still happens at 2048 granularity (one transfer per chain-pair,
    # issued with that chain's first group) to keep SP dispatch count down.
    CAD_A, CAD_V, V_OFF = 1.0, 2.4, 6.5
    tagged = [("A", i, g) for i, g in enumerate(act_groups)] + [
        ("V", i, g) for i, g in enumerate(dve_groups)
    ]
    tagged.sort(key=lambda t: (t[1] + 0.5) * CAD_A if t[0] == "A"
                else V_OFF + (t[1] + 0.5) * CAD_V)
    chain_end = {"A": cut, "V": n_pad}

    with tile.TileContext(nc) as tc:
        with (
            tc.tile_pool(name="const", bufs=1) as const_pool,
            tc.tile_pool(name="xrA", bufs=3) as xrA_pool,
            tc.tile_pool(name="xrV", bufs=3) as xrV_pool,
            tc.tile_pool(name="xrnA", bufs=3) as xrnA_pool,
            tc.tile_pool(name="xrnV", bufs=3) as xrnV_pool,
            tc.tile_pool(name="y0", bufs=2) as y0_pool,
            tc.tile_pool(name="sbig", bufs=1) as s_pool,
            tc.tile_pool(name="scr", bufs=2) as scr_pool,
            tc.tile_pool(name="psA", bufs=2, space="PSUM") as psA,
            tc.tile_pool(name="psV", bufs=2, space="PSUM") as psV,
        ):
            xT2 = const_pool.tile([D, B_LOC], F16)
            xn = const_pool.tile([128, 1], F32)
            seedc0 = const_pool.tile([128, 1], F32)
            seedc1 = const_pool.tile([128, 1], F32)
            ones = const_pool.tile([1, 128], F16)
            cls = const_pool.tile([128, C], F32)

            nc.gpsimd.memset(ones[:], 1.0)

            # warm-up: pull the Sqrt table load to t~0 (dependency-free)
            warm = const_pool.tile([128, 1], F32)
            nc.gpsimd.memset(warm[:], 1.0)
            nc.scalar.activation(
                warm[:], warm[:], mybir.ActivationFunctionType.Sqrt
            )

            s_sb = s_pool.tile([128, n_pad], F16)

            # ---- Phase 1: stream bank; matmuls; sqrt on ACT or DVE ----
            first = True
            stream = {"A": None, "V": None}   # chain -> (xr_t, xrn_t, base, end)
            for tag, _, (g0, g1) in tagged:
                w = g1 - g0
                st = stream[tag]
                if st is None or g0 >= st[3]:
                    pe = min(g0 + 2 * GROUP, chain_end[tag])
                    pw = pe - g0
                    xrnp = xrnA_pool if tag == "A" else xrnV_pool
                    xrp = xrA_pool if tag == "A" else xrV_pool
                    xrn_t = xrnp.tile([1, 2 * GROUP], F16, tag="xrn" + tag)
                    nc.sync.dma_start(xrn_t[:, :pw], xrn_d.ap()[:, g0:pe])
                    xr_t = xrp.tile([D, 2 * GROUP], F16, tag="xr" + tag)
                    nc.sync.dma_start(xr_t[:, :pw], xrT_d.ap()[:, g0:pe])
                    st = stream[tag] = (xr_t, xrn_t, g0, pe)
                    if first:
                        nc.sync.dma_start(xT2[:], xT2_d.ap())
                        nc.gpsimd.dma_start(xn[:], xn_d.ap())
                        nc.gpsimd.dma_start(seedc0[:], sc0_d.ap())
                        nc.gpsimd.dma_start(seedc1[:], sc1_d.ap())
                        first = False
                xr_t, xrn_t, base, _ = st
                q0 = g0 - base

                pool = psA if tag == "A" else psV
                d2 = pool.tile([128, GROUP], F32, tag="d2" + tag)
                for j0 in range(0, w, 512):
                    jw = min(512, w - j0)
                    nc.tensor.matmul(
                        d2[:, j0 : j0 + jw], ones[:],
                        xrn_t[:, q0 + j0 : q0 + j0 + jw],
                        start=True, stop=False,
                    )
                for j0 in range(0, w, 512):
                    jw = min(512, w - j0)
                    nc.tensor.matmul(
                        d2[:, j0 : j0 + jw], xT2[:],
                        xr_t[:, q0 + j0 : q0 + j0 + jw],
                        start=False, stop=True,
                    )
                if tag == "A":
                    nc.scalar.activation(
                        s_sb[:, g0 : g0 + w], d2[:, :w],
                        mybir.ActivationFunctionType.Sqrt,
                        bias=xn[:, 0:1], scale=1.0,
                    )
                else:
                    # DVE path: psum lacks ||x_b||^2 (the ACT chain adds
                    # it via the Sqrt bias); folded per-partition instead:
                    # seed coeffs are shifted polynomials in xnorm_b and the
                    # Newton op adds xnorm_b (seed_c0/c1, xn are [128,1]).
                    y0 = y0_pool.tile([128, GROUP], F32, tag="y0")
                    nc.vector._custom_dve(
                        RSQRT_SEED_ANT, out=y0[:, :w], in0=d2[:, :w],
                        s0=seedc0[:, 0:1], s1=seedc1[:, 0:1],
                        imm2=SEED_CONSTS[2],
                    )
                    nc.vector._custom_dve(
                        SQRT_NR_BIAS_ANT, out=s_sb[:, g0 : g0 + w],
                        in0=d2[:, :w], in1=y0[:, :w],
                        s0=xn[:, 0:1], s1=1.5, imm2=0.5,
                    )

            # ---- fence on ACT (Copy is in every table set): zeros tile from
            # the last ACT-chain Sqrt output; gates the Exp phase ordering
            fence0 = const_pool.tile([128, 1], F32)
            nc.scalar.mul(fence0[:], s_sb[:, cut - 1 : cut], 0.0)

            # ---- Phase 2: Exp with accumulate -> per-class sums ----
            # ACT-computed classes first (their s is ready and the table just
            # loaded); DVE classes follow as their s lands.
            order_k = sorted(range(K_CUT), key=lambda k: -caps[k]) + list(
                range(K_CUT, C)
            )
            for k in order_k:
                e_scr = scr_pool.tile([128, max_cap], F16, tag="escr")
                nc.scalar.activation(
                    e_scr[:, : caps[k]],
                    s_sb[:, offs[k] : offs[k + 1]],
                    mybir.ActivationFunctionType.Exp,
                    bias=fence0[:, 0:1],
                    scale=-1.0,
                    accum_out=cls[:, k : k + 1],
                )
            nc.sync.dma_start(cls_d.ap(), cls[:])

    nc.compile()
    return nc, {
        "xT2": xT2_d.name,
        "xrT": xrT_d.name,
        "xrnorm": xrn_d.name,
        "xnorm": xn_d.name,
        "seedc0": sc0_d.name,
        "seedc1": sc1_d.name,
        "cls": cls_d.name,
    }


def _prepare_inputs(x, x_ref, y_ref, caps):
    """Sorted/padded bank (shared) + per-core query blocks."""
    offs = np.concatenate([[0], np.cumsum(caps)]).astype(int)
    n_pad = int(offs[-1])

    x = np.asarray(x, dtype=np.float32)
    x_ref = np.asarray(x_ref, dtype=np.float32)
    y_ref = np.asarray(y_ref).astype(np.int64)

    xnorm = (x.astype(np.float64) ** 2).sum(axis=1).astype(np.float32)  # [B]
    xrnorm = (x_ref.astype(np.float64) ** 2).sum(axis=1).astype(np.float32)

    order = np.argsort(y_ref, kind="stable")
    counts = np.bincount(y_ref, minlength=C)
    xrT_pad = np.zeros((D, n_pad), dtype=np.float16)
    xrn_pad = np.full((1, n_pad), PAD_NORM, dtype=np.float16)
    pos = 0
    for k in range(C):
        cnt = int(counts[k])
        assert cnt <= caps[k], (k, cnt, caps[k])
        idx = order[pos : pos + cnt]
        pos += cnt
        xrT_pad[:, offs[k] : offs[k] + cnt] = x_ref[idx].T.astype(np.float16)
        xrn_pad[0, offs[k] : offs[k] + cnt] = xrnorm[idx].astype(np.float16)

    c0, c1, c2 = SEED_CONSTS
    blocks = []
    for i in range(N_CORES):
        sl = slice(i * B_LOC, (i + 1) * B_LOC)
        xT2 = (-2.0 * x[sl].T).astype(np.float16)  # [D, B_LOC]
        xb = xnorm[sl].astype(np.float64)
        xn_t = xnorm[sl].reshape(B_LOC, 1).copy()  # [128, 1]
        sc0 = (c0 + c1 * xb + c2 * xb * xb).reshape(B_LOC, 1).astype(np.float32)
        sc1 = (c1 + 2.0 * c2 * xb).reshape(B_LOC, 1).astype(np.float32)
        blocks.append((xT2, xn_t, sc0, sc1))

    return xrT_pad, xrn_pad, blocks


def kernel(x, x_ref, y, y_ref):
    x = np.asarray(x)
    x_ref = np.asarray(x_ref)
    y = np.asarray(y).astype(np.int64)
    y_ref_i = np.asarray(y_ref).astype(np.int64)

    counts = np.bincount(y_ref_i, minlength=C)
    caps = [max(16, ((int(c) + 15) // 16) * 16) for c in counts]

    key = tuple(caps)
    if key not in _MODULE_CACHE:
        _MODULE_CACHE[key] = _build_module(caps)
    nc, names = _MODULE_CACHE[key]
    xrT_pad, xrn_pad, blocks = _prepare_inputs(x, x_ref, y_ref_i, caps)

    in_maps = []
    for core in range(N_CORES):
        xT2, xn_t, sc0, sc1 = blocks[core]
        in_maps.append(
            {
                names["xT2"]: xT2,
                names["xrT"]: xrT_pad,
                names["xrnorm"]: xrn_pad,
                names["xnorm"]: xn_t,
                names["seedc0"]: sc0,
                names["seedc1"]: sc1,
            }
        )

    trace = bool(int(os.environ.get("KERNEL_TRACE", "0")))
    res = run_bass_kernel_spmd(
        nc, in_maps, core_ids=list(range(N_CORES)), trace=trace
    )
    LAST["exec_time_ns"] = res.exec_time_ns
    LAST["results"] = res
    LAST["module"] = nc

    # ---- host combine: concat per-core class sums, then NLL ----
    cs = np.concatenate(
        [np.asarray(res.results[core][names["cls"]], dtype=np.float64)
         for core in range(N_CORES)],
        axis=0,
    )  # [B, C]

    total = cs.sum(axis=1, keepdims=True)
    soft = cs / total + 1e-6
    loss = -np.mean(np.log(soft[np.arange(B), y]))
    return np.asarray(loss, dtype=np.float32)



# revision 10
# speedup vs baseline: 1.4804x; 1.4804x over previous
"""Soft-KNN NLL loss (ASKLoss) Trainium2 kernel — v2.

Problem: x[1024,128] queries vs x_ref[50000,128] bank,
  score = -||x - xr||_2, probs = softmax over the 50000 refs,
  soft_nns = probs @ onehot(y_ref) + 1e-6, loss = -mean(log(soft_nns[b, y[b]])).

Data-parallel over the query batch across 8 cores (128 queries/core).

Per core (v2 design):
  - d2 via ONE fp8e4 DoubleRow matmul pass: K_phys=67 partitions x 2 k-tiles.
    Partitions 0..63 carry the 128 xr dims (2 per partition); partitions
    64..66 carry multi-limb fp8 encodings of xrnorm-160 (moving side) and
    (xnorm+160)/2 (stationary side, via ones columns), so psum = full d2.
    Cost halves vs fp16 (0.5 PE cycles/col) and there is no rank-1 pass.
  - refs are class-sorted and split into an ACT region and a DVE region
    (fraction F1 to ACT); per-query weight w = exp(16 - sqrt(d2)):
      ACT region: Sqrt(psum) -> s fp16 (phase 1), then per-class
        Exp(16 - s) with accum_out -> class partial sums (phase 2; one
        table switch between phases, Identity-fence enforces order).
      DVE region: one fused custom op (rsqrt seed + Newton) -> st = s/2.598
        fp16, then one fused custom op Q8: quadratic Q(st) ~ exp((16-s)/8),
        out Q^8 with accum -> class partial sums.  Per-element weight errors
        up to ~40% are smooth in s and cancel in the softmax ratio (host
        rehearsal: loss rel err ~2e-5 vs the 2e-2 budget).
  - groups of 1024 cols stream through PSUM (2+2 tiles = 8 banks),
    cadence-interleaved by per-engine consumption rate.

Host: concat per-core class partials, NLL in f64.
"""

import os
import re

import numpy as np
import ml_dtypes

import concourse.bass as bass
import concourse.dve_ops as dops
import concourse.mybir as mybir
import concourse.tile as tile
from concourse import bacc
from concourse.bass_utils import run_bass_kernel_spmd
from concourse.dve_spec import C0, C1, C2, Spec, Src0, AluOp, One, sq

B, N, D, C = 1024, 50000, 128, 10
N_CORES = 8
B_LOC = B // N_CORES
GROUP = 1024
KP = 67                       # 64 data partitions + 3 limb partitions
F1 = 0.53                     # fraction of columns on the ACT path
CEXP = 16.0                   # global exp centering: w = exp(CEXP - s)
SQ3 = 1.7320508075688772
NEWTON = 2.598076211353316    # s = NEWTON * st

F8 = mybir.dt.float8e4
F16 = mybir.dt.float16
F32 = mybir.dt.float32
NP8 = ml_dtypes.float8_e4m3

LAST = {}
_MODULE_CACHE = {}

# ---- custom DVE ops --------------------------------------------------------


def _register_op(name, spec_body, ref, accum=None):
    if name in dops._SUB_OPCODE_FOR_NAME:
        for op in dops.OPS:
            if op.name == name:
                return op
    spec = (Spec(body=spec_body, reference=ref, accum=accum)
            if accum else Spec(body=spec_body, reference=ref))
    probe = dops.DveOp(name, spec, subdim=False, uops_sha={})
    dops.OPS.append(probe)
    dops._SUB_OPCODE_FOR_NAME[name] = (
        dops._CUSTOM_DVE_ROW_BASE + len(dops.OPS) - 1
    )
    assert dops._SUB_OPCODE_FOR_NAME[name] < 0x20
    shas = {}
    for ver in ("v3", "v4"):
        try:
            probe.compile(ver)
            shas[ver] = probe.uops_sha.get(ver)
        except ValueError as e:
            shas[ver] = re.search(r'="([0-9a-f]+)"', str(e)).group(1)
    final = dops.DveOp(name, spec, subdim=False, uops_sha=shas)
    dops.OPS[-1] = final
    dops.CUSTOM_DVE_SPECS[name] = final.spec
    return final


# op1: h = C0 + u C1 + u^2 C2  (~ rsqrt(u)/sqrt(3)); out = t(1 - t h), t = u h
# => out = sqrt(u)/2.598 after one Newton step (exact 8 ALU stages).
_h = C0 + Src0 * (C1 + Src0 * C2)
_t = Src0 * _h


def _ref_op1(in0, in1, c0, c1, c2):
    h = c0 + in0 * (c1 + in0 * c2)
    t = in0 * h
    return t * (1.0 - t * h)


OP1 = _register_op("SQRT_FUSED_ANT", _t * (One - _t * _h), _ref_op1)

# op2: Q = C0 + st C1 + st^2 C2 (~ exp((CEXP - NEWTON*st)/8)); out = Q^8,
# accum_out = row-sum of out (4 + 3 + accum = 8 ALU stages).
_Q = C0 + Src0 * (C1 + Src0 * C2)


def _ref_op2(in0, in1, c0, c1, c2):
    q = c0 + in0 * (c1 + in0 * c2)
    return ((q * q) ** 2) ** 2


OP2 = _register_op("EXPQ8_ACC_ANT", sq(sq(sq(_Q))), _ref_op2, accum=AluOp.ADD)


# ---- host-side fits --------------------------------------------------------


def _fit_rel(f, lo, hi, deg, npts=4001, iters=10):
    u = np.linspace(lo, hi, npts)
    t = f(u)
    w = 1.0 / np.abs(t)
    V = np.vander(u, deg + 1, increasing=True)
    c = None
    for _ in range(iters):
        c = np.linalg.lstsq(V * w[:, None], t * w, rcond=None)[0]
        r = np.abs((V @ c - t) / t)
        w = w * (0.5 + r / r.max())
    return c


def _coeffs(u_lo, u_hi):
    ch = _fit_rel(lambda u: 1.0 / np.sqrt(u) / SQ3, u_lo, u_hi, 2)
    st_lo = np.sqrt(u_lo) / NEWTON - 0.05
    st_hi = np.sqrt(u_hi) / NEWTON + 0.05
    cq = _fit_rel(lambda v: np.exp((CEXP - NEWTON * v) / 8.0), st_lo, st_hi, 2)
    return tuple(float(v) for v in ch), tuple(float(v) for v in cq)


# ---- module build ----------------------------------------------------------


def _build_module(n_pad, a_spans, v_spans, ch, cq):
    """a_spans/v_spans: per-class (start, end) column spans (absolute)."""
    n_A = a_spans[-1][1] if a_spans else 0

    nc = bacc.Bacc(
        "TRN2",
        target_bir_lowering=False,
        debug=False,
        enable_asserts=True,
        num_devices=N_CORES,
    )

    wts_d = nc.dram_tensor("wts", [KP, 2, B_LOC], F8, kind="ExternalInput")
    xrp_d = nc.dram_tensor("xrp", [KP, 2, n_pad], F8, kind="ExternalInput")
    cls_d = nc.dram_tensor("cls", [B_LOC, 2 * C], F32, kind="ExternalOutput")
    debug = bool(int(os.environ.get("KERNEL_DEBUG", "0")))
    if debug:
        sdump_d = nc.dram_tensor("sdump", [B_LOC, n_pad], F16,
                                 kind="ExternalOutput")

    n_groups = n_pad // GROUP
    assert n_pad % GROUP == 0

    # cadence interleave: ACT consumes an A-group every ~1.0us (phase 1);
    # DVE consumes a V-group every ~2.28us (op1+op2 amortized).
    a_groups = [g for g in range(n_groups) if g * GROUP < n_A]
    v_groups = [g for g in range(n_groups) if g * GROUP >= n_A]
    CAD_A, CAD_V = 1.0, 2.28
    tagged = [("A", i, g) for i, g in enumerate(a_groups)] + [
        ("V", i, g) for i, g in enumerate(v_groups)
    ]
    tagged.sort(key=lambda t: (t[1] + 0.5) * (CAD_A if t[0] == "A" else CAD_V))

    with tile.TileContext(nc) as tc:
        with (
            tc.tile_pool(name="const", bufs=1) as const_pool,
            tc.tile_pool(name="xrA", bufs=3) as xrA_pool,
            tc.tile_pool(name="xrV", bufs=3) as xrV_pool,
            tc.tile_pool(name="sbig", bufs=1) as s_pool,
            tc.tile_pool(name="scrA", bufs=2) as scrA_pool,
            tc.tile_pool(name="scrV", bufs=2) as scrV_pool,
            tc.tile_pool(name="psA", bufs=2, space="PSUM") as psA,
            tc.tile_pool(name="psV", bufs=2, space="PSUM") as psV,
        ):
            wt = const_pool.tile([KP, 2, B_LOC], F8)
            cls = const_pool.tile([B_LOC, 2 * C], F32)
            nc.gpsimd.memset(cls[:], 0.0)

            # warm-up: pull the Sqrt table load to t~0
            warm = const_pool.tile([128, 1], F32)
            nc.gpsimd.memset(warm[:], 1.0)
            nc.scalar.activation(
                warm[:], warm[:], mybir.ActivationFunctionType.Sqrt
            )
            c16 = const_pool.tile([B_LOC, 1], F32)
            nc.gpsimd.memset(c16[:], CEXP)

            s_sb = s_pool.tile([B_LOC, n_pad], F16)

            # ---- phase 1: stream bank, matmul, sqrt (ACT) / op1 (DVE) ----
            first = True
            stream = {"A": None, "V": None}
            v_emitted = 0
            pend_last_sqrt = [None]

            def emit_v_chunks(upto):
                nonlocal v_emitted
                while v_emitted < len(v_spans) and v_spans[v_emitted][1] <= upto:
                    a, b = v_spans[v_emitted]
                    k = v_emitted
                    scr = scrV_pool.tile([B_LOC, 4096], F16, tag="scrV")
                    nc.vector._custom_dve(
                        OP2, out=scr[:, : b - a], in0=s_sb[:, a:b],
                        s0=cq[0], s1=cq[1], imm2=cq[2],
                        accum_out=cls[:, C + k : C + k + 1],
                    )
                    v_emitted += 1

            for tag, _, g in tagged:
                g0, g1 = g * GROUP, (g + 1) * GROUP
                st_ = stream[tag]
                if st_ is None or g0 >= st_[2]:
                    pe = min(g0 + 2 * GROUP, n_pad)
                    if tag == "A":
                        pe = min(pe, n_A)
                    xrp = xrA_pool if tag == "A" else xrV_pool
                    xr_t = xrp.tile([KP, 2, 2 * GROUP], F8, tag="xr" + tag)
                    nc.sync.dma_start(
                        xr_t[:, :, : pe - g0], xrp_d.ap()[:, :, g0:pe]
                    )
                    st_ = stream[tag] = (xr_t, g0, pe)
                    if first:
                        nc.sync.dma_start(wt[:], wts_d.ap())
                        first = False
                xr_t, base, _ = st_
                q0 = g0 - base

                pool = psA if tag == "A" else psV
                d2 = pool.tile([B_LOC, GROUP], F32, tag="d2" + tag)
                for j in range(0, GROUP, 512):
                    nc.tensor.matmul(
                        d2[:, j : j + 512], wt[:],
                        xr_t[:, :, q0 + j : q0 + j + 512],
                        start=True, stop=True,
                        perf_mode=mybir.MatmulPerfMode.DoubleRow,
                    )
                if tag == "A":
                    nc.scalar.activation(
                        s_sb[:, g0:g1], d2[:, :GROUP],
                        mybir.ActivationFunctionType.Sqrt,
                    )
                    pend_last_sqrt[0] = g1
                else:
                    nc.vector._custom_dve(
                        OP1, out=s_sb[:, g0:g1], in0=d2[:, :GROUP],
                        s0=ch[0], s1=ch[1], imm2=ch[2],
                    )
                    emit_v_chunks(g1)

            # ---- fence: Identity(0*s_last + CEXP) -> bias tile; orders the
            # Exp table load after every Sqrt on the in-order ACT queue.
            fence = const_pool.tile([B_LOC, 1], F32)
            last = pend_last_sqrt[0] or 1
            nc.scalar.activation(
                fence[:], s_sb[:, last - 1 : last],
                mybir.ActivationFunctionType.Identity,
                scale=0.0, bias=c16[:, 0:1],
            )

            # ---- phase 2: per-class Exp with accumulate (ACT region) ----
            for k, (a, b) in enumerate(a_spans):
                if b <= a:
                    continue
                scr = scrA_pool.tile([B_LOC, 4096], F16, tag="scrA")
                nc.scalar.activation(
                    scr[:, : b - a], s_sb[:, a:b],
                    mybir.ActivationFunctionType.Exp,
                    bias=fence[:, 0:1], scale=-1.0,
                    accum_out=cls[:, k : k + 1],
                )
            nc.sync.dma_start(cls_d.ap(), cls[:])
            if debug:
                nc.sync.dma_start(sdump_d.ap(), s_sb[:])

    nc.compile()
    return nc, {"wts": wts_d.name, "xrp": xrp_d.name, "cls": cls_d.name}


# ---- host prep -------------------------------------------------------------


def _f8(a):
    return np.asarray(a, dtype=NP8)


def _limbs3(v, s1=16.0, s2=256.0):
    """v ~ h0 + h1/s1 + h2/s2 with fp8 limbs."""
    h0 = _f8(v)
    r1 = v - h0.astype(np.float64)
    h1 = _f8(s1 * r1)
    r2 = r1 - h1.astype(np.float64) / s1
    h2 = _f8(s2 * r2)
    return h0, h1, h2


def _prepare(x, x_ref, y_ref):
    x = np.asarray(x, dtype=np.float32)
    x_ref = np.asarray(x_ref, dtype=np.float32)
    y_ref = np.asarray(y_ref).astype(np.int64)

    xnorm = (x.astype(np.float64) ** 2).sum(axis=1)
    xrnorm = (x_ref.astype(np.float64) ** 2).sum(axis=1)

    counts = np.bincount(y_ref, minlength=C)
    order = np.argsort(y_ref, kind="stable")

    # region split: per class, first nA_k cols -> ACT region, rest -> DVE.
    # Force the A-region total onto a GROUP boundary (no straddle groups:
    # every psum group is consumed by exactly one engine).
    nA_k = [int(round(F1 * int(c))) for c in counts]
    r = sum(nA_k) % GROUP
    adj = GROUP - r if r > GROUP // 2 else -r
    for k in np.argsort([-c for c in counts]):
        take = int(np.clip(nA_k[k] + adj, 0, int(counts[k]))) - nA_k[k]
        nA_k[k] += take
        adj -= take
        if adj == 0:
            break
    assert sum(nA_k) % GROUP == 0
    a_spans, v_spans = [], []
    pos = 0
    a_idx, v_idx = [], []
    for k in range(C):
        cls_idx = order[pos : pos + int(counts[k])]
        pos += int(counts[k])
        a_idx.append(cls_idx[: nA_k[k]])
        v_idx.append(cls_idx[nA_k[k] :])
    col = 0
    for k in range(C):
        a_spans.append((col, col + len(a_idx[k])))
        col += len(a_idx[k])
    for k in range(C):
        v_spans.append((col, col + len(v_idx[k])))
        col += len(v_idx[k])
    n_real = col
    n_pad = ((n_real + GROUP - 1) // GROUP) * GROUP
    perm = np.concatenate(a_idx + v_idx)

    # moving-side pack [KP, 2, n_pad]
    xrp = np.zeros((KP, 2, n_pad), dtype=NP8)
    xrs = x_ref[perm].T.astype(np.float64)  # [D, n_real]
    for k in range(64):
        xrp[k, 0, :n_real] = _f8(xrs[k])
        xrp[k, 1, :n_real] = _f8(xrs[64 + k])
    g = np.full(n_pad, 240.0)  # pads: xrnorm ~ 400 -> s ~ 21.5+, w ~ e-6
    g[:n_real] = xrnorm[perm] - 160.0
    g0, g1_, g2 = _limbs3(g)
    xrp[64, 0, :] = g0
    xrp[64, 1, :] = g1_
    xrp[65, 0, :] = g2
    xrp[65, 1, :] = _f8(2.0)
    xrp[66, 0, :] = _f8(1.0 / 8.0)
    xrp[66, 1, :] = _f8(1.0 / 128.0)

    # stationary packs per core [KP, 2, B_LOC]
    blocks = []
    for i in range(N_CORES):
        sl = slice(i * B_LOC, (i + 1) * B_LOC)
        xb = x[sl].astype(np.float64)  # [128, D]
        wts = np.zeros((KP, 2, B_LOC), dtype=NP8)
        for k in range(64):
            wts[k, 0, :] = _f8(-2.0 * xb[:, k])
            wts[k, 1, :] = _f8(-2.0 * xb[:, 64 + k])
        wts[64, 0, :] = _f8(1.0)
        wts[64, 1, :] = _f8(1.0 / 16.0)
        wts[65, 0, :] = _f8(1.0 / 256.0)
        hh = (xnorm[sl] + 160.0) / 2.0
        h0, h1, h2 = _limbs3(hh)
        wts[65, 1, :] = h0
        wts[66, 0, :] = h1
        wts[66, 1, :] = h2
        blocks.append(wts)

    # u-range via sampling (data is what it is; pad margin for fp8 noise)
    samp = x[:: max(1, B // 64)].astype(np.float32)
    d2s = (
        (samp.astype(np.float64) ** 2).sum(1)[:, None]
        + xrnorm[None, :]
        - 2.0 * samp.astype(np.float64) @ x_ref.T.astype(np.float64)
    )
    # fit on the (sampled) real-data range only; pads land outside but are
    # never accumulated — op1 merely has to stay finite there (it does).
    u_lo = max(1.0, d2s.min() - 15.0)
    u_hi = d2s.max() + 15.0

    return xrp, blocks, a_spans, v_spans, n_pad, u_lo, u_hi


def kernel(x, x_ref, y, y_ref):
    x = np.asarray(x)
    x_ref = np.asarray(x_ref)
    y = np.asarray(y).astype(np.int64)
    y_ref_i = np.asarray(y_ref).astype(np.int64)

    xrp, blocks, a_spans, v_spans, n_pad, u_lo, u_hi = _prepare(
        x, x_ref, y_ref_i
    )
    ch, cq = _coeffs(u_lo, u_hi)

    key = (n_pad, tuple(a_spans), tuple(v_spans), ch, cq)
    if key not in _MODULE_CACHE:
        _MODULE_CACHE[key] = _build_module(n_pad, a_spans, v_spans, ch, cq)
    nc, names = _MODULE_CACHE[key]

    in_maps = [
        {names["wts"]: blocks[core], names["xrp"]: xrp}
        for core in range(N_CORES)
    ]

    trace = bool(int(os.environ.get("KERNEL_TRACE", "0")))
    res = run_bass_kernel_spmd(
        nc, in_maps, core_ids=list(range(N_CORES)), trace=trace
    )
    LAST["exec_time_ns"] = res.exec_time_ns
    LAST["results"] = res
    LAST["module"] = nc

    cs_parts = []
    for core in range(N_CORES):
        cl = np.asarray(res.results[core][names["cls"]], dtype=np.float64)
        cs_parts.append(cl[:, :C] + cl[:, C:])
    cs = np.concatenate(cs_parts, axis=0)  # [B, C]

    total = cs.sum(axis=1, keepdims=True)
    soft = cs / total + 1e-6
    loss = -np.mean(np.log(soft[np.arange(B), y]))
    return np.asarray(loss, dtype=np.float32)


# revision 13
# speedup vs baseline: 1.4931x; 1.0086x over previous
"""Soft-KNN NLL loss (ASKLoss) Trainium2 kernel — v2.

Problem: x[1024,128] queries vs x_ref[50000,128] bank,
  score = -||x - xr||_2, probs = softmax over the 50000 refs,
  soft_nns = probs @ onehot(y_ref) + 1e-6, loss = -mean(log(soft_nns[b, y[b]])).

Data-parallel over the query batch across 8 cores (128 queries/core).

Per core (v2 design):
  - d2 via ONE fp8e4 DoubleRow matmul pass: K_phys=67 partitions x 2 k-tiles.
    Partitions 0..63 carry the 128 xr dims (2 per partition); partitions
    64..66 carry multi-limb fp8 encodings of xrnorm-160 (moving side) and
    (xnorm+160)/2 (stationary side, via ones columns), so psum = full d2.
    Cost halves vs fp16 (0.5 PE cycles/col) and there is no rank-1 pass.
  - refs are class-sorted and split into an ACT region and a DVE region
    (fraction F1 to ACT); per-query weight w = exp(16 - sqrt(d2)):
      ACT region: Sqrt(psum) -> s fp16 (phase 1), then per-class
        Exp(16 - s) with accum_out -> class partial sums (phase 2; one
        table switch between phases, Identity-fence enforces order).
      DVE region: one fused custom op (rsqrt seed + Newton) -> st = s/2.598
        fp16, then one fused custom op Q8: quadratic Q(st) ~ exp((16-s)/8),
        out Q^8 with accum -> class partial sums.  Per-element weight errors
        up to ~40% are smooth in s and cancel in the softmax ratio (host
        rehearsal: loss rel err ~2e-5 vs the 2e-2 budget).
  - groups of 1024 cols stream through PSUM (2+2 tiles = 8 banks),
    cadence-interleaved by per-engine consumption rate.

Host: concat per-core class partials, NLL in f64.
"""

import os
import re

import numpy as np
import ml_dtypes

import concourse.bass as bass
import concourse.dve_ops as dops
import concourse.mybir as mybir
import concourse.tile as tile
from concourse import bacc
from concourse.bass_utils import run_bass_kernel_spmd
from concourse.dve_spec import C0, C1, C2, Spec, Src0, AluOp, One, sq

B, N, D, C = 1024, 50000, 128, 10
N_CORES = 8
B_LOC = B // N_CORES
GROUP = 1024
KP = 67                       # 64 data partitions + 3 limb partitions
F1 = 0.53                     # fraction of columns on the ACT path
CEXP = 16.0                   # global exp centering: w = exp(CEXP - s)
SQ3 = 1.7320508075688772
NEWTON = 2.598076211353316    # s = NEWTON * st

F8 = mybir.dt.float8e4
F16 = mybir.dt.float16
F32 = mybir.dt.float32
NP8 = ml_dtypes.float8_e4m3

LAST = {}
_MODULE_CACHE = {}

# ---- custom DVE ops --------------------------------------------------------


def _register_op(name, spec_body, ref, accum=None):
    if name in dops._SUB_OPCODE_FOR_NAME:
        for op in dops.OPS:
            if op.name == name:
                return op
    spec = (Spec(body=spec_body, reference=ref, accum=accum)
            if accum else Spec(body=spec_body, reference=ref))
    probe = dops.DveOp(name, spec, subdim=False, uops_sha={})
    dops.OPS.append(probe)
    dops._SUB_OPCODE_FOR_NAME[name] = (
        dops._CUSTOM_DVE_ROW_BASE + len(dops.OPS) - 1
    )
    assert dops._SUB_OPCODE_FOR_NAME[name] < 0x20
    shas = {}
    for ver in ("v3", "v4"):
        try:
            probe.compile(ver)
            shas[ver] = probe.uops_sha.get(ver)
        except ValueError as e:
            shas[ver] = re.search(r'="([0-9a-f]+)"', str(e)).group(1)
    final = dops.DveOp(name, spec, subdim=False, uops_sha=shas)
    dops.OPS[-1] = final
    dops.CUSTOM_DVE_SPECS[name] = final.spec
    return final


# op1: h = C0 + u C1 + u^2 C2  (~ rsqrt(u)/sqrt(3)); out = t(1 - t h), t = u h
# => out = sqrt(u)/2.598 after one Newton step (exact 8 ALU stages).
_h = C0 + Src0 * (C1 + Src0 * C2)
_t = Src0 * _h


def _ref_op1(in0, in1, c0, c1, c2):
    h = c0 + in0 * (c1 + in0 * c2)
    t = in0 * h
    return t * (1.0 - t * h)


OP1 = _register_op("SQRT_FUSED_ANT", _t * (One - _t * _h), _ref_op1)

# op2: Q = C0 + st C1 + st^2 C2 (~ exp((CEXP - NEWTON*st)/8)); out = Q^8,
# accum_out = row-sum of out (4 + 3 + accum = 8 ALU stages).
_Q = C0 + Src0 * (C1 + Src0 * C2)


def _ref_op2(in0, in1, c0, c1, c2):
    q = c0 + in0 * (c1 + in0 * c2)
    return ((q * q) ** 2) ** 2


OP2 = _register_op("EXPQ8_ACC_ANT", sq(sq(sq(_Q))), _ref_op2, accum=AluOp.ADD)


# ---- host-side fits --------------------------------------------------------


def _fit_rel(f, lo, hi, deg, npts=4001, iters=10):
    u = np.linspace(lo, hi, npts)
    t = f(u)
    w = 1.0 / np.abs(t)
    V = np.vander(u, deg + 1, increasing=True)
    c = None
    for _ in range(iters):
        c = np.linalg.lstsq(V * w[:, None], t * w, rcond=None)[0]
        r = np.abs((V @ c - t) / t)
        w = w * (0.5 + r / r.max())
    return c


def _coeffs(u_lo, u_hi):
    ch = _fit_rel(lambda u: 1.0 / np.sqrt(u) / SQ3, u_lo, u_hi, 2)
    st_lo = np.sqrt(u_lo) / NEWTON - 0.05
    st_hi = np.sqrt(u_hi) / NEWTON + 0.05
    cq = _fit_rel(lambda v: np.exp((CEXP - NEWTON * v) / 8.0), st_lo, st_hi, 2)
    return tuple(float(v) for v in ch), tuple(float(v) for v in cq)


# ---- module build ----------------------------------------------------------


def _build_module(n_pad, a_spans, v_spans, ch, cq):
    """a_spans/v_spans: per-class (start, end) column spans (absolute)."""
    n_A = a_spans[-1][1] if a_spans else 0

    nc = bacc.Bacc(
        "TRN2",
        target_bir_lowering=False,
        debug=False,
        enable_asserts=True,
        num_devices=N_CORES,
    )

    wts_d = nc.dram_tensor("wts", [KP, 2, B_LOC], F8, kind="ExternalInput")
    xrp_d = nc.dram_tensor("xrp", [KP, 2, n_pad], F8, kind="ExternalInput")
    cls_d = nc.dram_tensor("cls", [B_LOC, 2 * C + 2], F32, kind="ExternalOutput")
    debug = bool(int(os.environ.get("KERNEL_DEBUG", "0")))
    if debug:
        sdump_d = nc.dram_tensor("sdump", [B_LOC, n_pad], F16,
                                 kind="ExternalOutput")

    n_groups = n_pad // GROUP
    assert n_pad % GROUP == 0

    # cadence interleave: ACT consumes an A-group every ~1.0us (phase 1);
    # DVE consumes a V-group every ~2.28us (op1+op2 amortized).
    a_groups = [g for g in range(n_groups) if g * GROUP < n_A]
    v_groups = [g for g in range(n_groups) if g * GROUP >= n_A]
    CAD_A, CAD_V = 1.04, 2.28
    tagged = [("A", i, g) for i, g in enumerate(a_groups)] + [
        ("V", i, g) for i, g in enumerate(v_groups)
    ]
    # V offset negative: DVE historically started ~4us late; front-load its
    # stream so the engine never starves (psV depth bounds the lead anyway).
    tagged.sort(key=lambda t: (t[1] + 0.5) * CAD_A if t[0] == "A"
                else (t[1] + 0.5) * CAD_V - 2.5)

    with tile.TileContext(nc) as tc:
        with (
            tc.tile_pool(name="const", bufs=1) as const_pool,
            tc.tile_pool(name="xrA", bufs=3) as xrA_pool,
            tc.tile_pool(name="xrV", bufs=3) as xrV_pool,
            tc.tile_pool(name="sbig", bufs=1) as s_pool,
            tc.tile_pool(name="scrA", bufs=2) as scrA_pool,
            tc.tile_pool(name="scrV", bufs=2) as scrV_pool,
            tc.tile_pool(name="psA", bufs=2, space="PSUM") as psA,
            tc.tile_pool(name="psV", bufs=2, space="PSUM") as psV,
        ):
            wt = const_pool.tile([KP, 2, B_LOC], F8)
            cls = const_pool.tile([B_LOC, 2 * C + 2], F32)
            nc.gpsimd.memset(cls[:], 0.0)

            # warm-up: pull the Sqrt table load to t~0
            warm = const_pool.tile([128, 1], F32)
            nc.gpsimd.memset(warm[:], 1.0)
            nc.scalar.activation(
                warm[:], warm[:], mybir.ActivationFunctionType.Sqrt
            )
            c16 = const_pool.tile([B_LOC, 1], F32)
            nc.gpsimd.memset(c16[:], CEXP)

            s_sb = s_pool.tile([B_LOC, n_pad], F16)

            # ---- phase 1: stream bank, matmul, sqrt (ACT) / op1 (DVE) ----
            nc.sync.dma_start(wt[:], wts_d.ap())
            first = True
            stream = {"A": None, "V": None}
            v_emitted = 0
            pend_last_sqrt = [None]

            # last-class V chunk split into thirds (extra accum cols) so the
            # post-stream DVE tail is ~0.8us instead of ~2.4us
            v_parts = []
            for k, (a, b) in enumerate(v_spans):
                if k == len(v_spans) - 1 and b - a > 900:
                    t1 = a + (b - a) // 3
                    t2 = a + 2 * (b - a) // 3
                    v_parts += [(a, t1, k), (t1, t2, 2 * C + 0), (t2, b, 2 * C + 1)]
                else:
                    v_parts.append((a, b, k))

            def emit_v_chunks(upto):
                nonlocal v_emitted
                while v_emitted < len(v_parts) and v_parts[v_emitted][1] <= upto:
                    a, b, slot = v_parts[v_emitted]
                    scr = scrV_pool.tile([B_LOC, 4096], F16, tag="scrV")
                    col = C + slot if slot < C else slot
                    nc.vector._custom_dve(
                        OP2, out=scr[:, : b - a], in0=s_sb[:, a:b],
                        s0=cq[0], s1=cq[1], imm2=cq[2],
                        accum_out=cls[:, col : col + 1],
                    )
                    v_emitted += 1

            for tag, _, g in tagged:
                g0, g1 = g * GROUP, (g + 1) * GROUP
                st_ = stream[tag]
                if st_ is None or g0 >= st_[2]:
                    pe = min(g0 + 2 * GROUP, n_pad)
                    if tag == "A":
                        pe = min(pe, n_A)
                    xrp = xrA_pool if tag == "A" else xrV_pool
                    xr_t = xrp.tile([KP, 2, 2 * GROUP], F8, tag="xr" + tag)
                    nc.sync.dma_start(
                        xr_t[:, :, : pe - g0], xrp_d.ap()[:, :, g0:pe]
                    )
                    st_ = stream[tag] = (xr_t, g0, pe)
                xr_t, base, _ = st_
                q0 = g0 - base

                pool = psA if tag == "A" else psV
                d2 = pool.tile([B_LOC, GROUP], F32, tag="d2" + tag)
                for j in range(0, GROUP, 512):
                    nc.tensor.matmul(
                        d2[:, j : j + 512], wt[:],
                        xr_t[:, :, q0 + j : q0 + j + 512],
                        start=True, stop=True,
                        perf_mode=mybir.MatmulPerfMode.DoubleRow,
                    )
                if tag == "A":
                    nc.scalar.activation(
                        s_sb[:, g0:g1], d2[:, :GROUP],
                        mybir.ActivationFunctionType.Sqrt,
                    )
                    pend_last_sqrt[0] = g1
                else:
                    nc.vector._custom_dve(
                        OP1, out=s_sb[:, g0:g1], in0=d2[:, :GROUP],
                        s0=ch[0], s1=ch[1], imm2=ch[2],
                    )
                    emit_v_chunks(g1)

            # ---- fence: Identity(0*s_last + CEXP) -> bias tile; orders the
            # Exp table load after every Sqrt on the in-order ACT queue.
            fence = const_pool.tile([B_LOC, 1], F32)
            last = pend_last_sqrt[0] or 1
            nc.scalar.activation(
                fence[:], s_sb[:, last - 1 : last],
                mybir.ActivationFunctionType.Identity,
                scale=0.0, bias=c16[:, 0:1],
            )

            # ---- phase 2: per-class Exp with accumulate (ACT region) ----
            for k, (a, b) in enumerate(a_spans):
                if b <= a:
                    continue
                scr = scrA_pool.tile([B_LOC, 4096], F16, tag="scrA")
                nc.scalar.activation(
                    scr[:, : b - a], s_sb[:, a:b],
                    mybir.ActivationFunctionType.Exp,
                    bias=fence[:, 0:1], scale=-1.0,
                    accum_out=cls[:, k : k + 1],
                )
            nc.gpsimd.dma_start(cls_d.ap(), cls[:])
            if debug:
                nc.sync.dma_start(sdump_d.ap(), s_sb[:])

    nc.compile()
    return nc, {"wts": wts_d.name, "xrp": xrp_d.name, "cls": cls_d.name}


# ---- host prep -------------------------------------------------------------


def _f8(a):
    return np.asarray(a, dtype=NP8)


def _limbs3(v, s1=16.0, s2=256.0):
    """v ~ h0 + h1/s1 + h2/s2 with fp8 limbs."""
    h0 = _f8(v)
    r1 = v - h0.astype(np.float64)
    h1 = _f8(s1 * r1)
    r2 = r1 - h1.astype(np.float64) / s1
    h2 = _f8(s2 * r2)
    return h0, h1, h2


def _prepare(x, x_ref, y_ref):
    x = np.asarray(x, dtype=np.float32)
    x_ref = np.asarray(x_ref, dtype=np.float32)
    y_ref = np.asarray(y_ref).astype(np.int64)

    xnorm = (x.astype(np.float64) ** 2).sum(axis=1)
    xrnorm = (x_ref.astype(np.float64) ** 2).sum(axis=1)

    counts = np.bincount(y_ref, minlength=C)
    order = np.argsort(y_ref, kind="stable")

    # region split: per class, first nA_k cols -> ACT region, rest -> DVE.
    # Force the A-region total onto a GROUP boundary (no straddle groups:
    # every psum group is consumed by exactly one engine).
    nA_k = [int(round(F1 * int(c))) for c in counts]
    r = sum(nA_k) % GROUP
    adj = GROUP - r if r > GROUP // 2 else -r
    for k in np.argsort([-c for c in counts]):
        take = int(np.clip(nA_k[k] + adj, 0, int(counts[k]))) - nA_k[k]
        nA_k[k] += take
        adj -= take
        if adj == 0:
            break
    assert sum(nA_k) % GROUP == 0
    a_spans, v_spans = [], []
    pos = 0
    a_idx, v_idx = [], []
    for k in range(C):
        cls_idx = order[pos : pos + int(counts[k])]
        pos += int(counts[k])
        a_idx.append(cls_idx[: nA_k[k]])
        v_idx.append(cls_idx[nA_k[k] :])
    col = 0
    for k in range(C):
        a_spans.append((col, col + len(a_idx[k])))
        col += len(a_idx[k])
    for k in range(C):
        v_spans.append((col, col + len(v_idx[k])))
        col += len(v_idx[k])
    n_real = col
    n_pad = ((n_real + GROUP - 1) // GROUP) * GROUP
    perm = np.concatenate(a_idx + v_idx)

    # moving-side pack [KP, 2, n_pad]
    xrp = np.zeros((KP, 2, n_pad), dtype=NP8)
    xrs = x_ref[perm].T.astype(np.float64)  # [D, n_real]
    for k in range(64):
        xrp[k, 0, :n_real] = _f8(xrs[k])
        xrp[k, 1, :n_real] = _f8(xrs[64 + k])
    g = np.full(n_pad, 240.0)  # pads: xrnorm ~ 400 -> s ~ 21.5+, w ~ e-6
    g[:n_real] = xrnorm[perm] - 160.0
    g0, g1_, g2 = _limbs3(g)
    xrp[64, 0, :] = g0
    xrp[64, 1, :] = g1_
    xrp[65, 0, :] = g2
    xrp[65, 1, :] = _f8(2.0)
    xrp[66, 0, :] = _f8(1.0 / 8.0)
    xrp[66, 1, :] = _f8(1.0 / 128.0)

    # stationary packs per core [KP, 2, B_LOC]
    blocks = []
    for i in range(N_CORES):
        sl = slice(i * B_LOC, (i + 1) * B_LOC)
        xb = x[sl].astype(np.float64)  # [128, D]
        wts = np.zeros((KP, 2, B_LOC), dtype=NP8)
        for k in range(64):
            wts[k, 0, :] = _f8(-2.0 * xb[:, k])
            wts[k, 1, :] = _f8(-2.0 * xb[:, 64 + k])
        wts[64, 0, :] = _f8(1.0)
        wts[64, 1, :] = _f8(1.0 / 16.0)
        wts[65, 0, :] = _f8(1.0 / 256.0)
        hh = (xnorm[sl] + 160.0) / 2.0
        h0, h1, h2 = _limbs3(hh)
        wts[65, 1, :] = h0
        wts[66, 0, :] = h1
        wts[66, 1, :] = h2
        blocks.append(wts)

    # u-range via sampling (data is what it is; pad margin for fp8 noise)
    samp = x[:: max(1, B // 64)].astype(np.float32)
    d2s = (
        (samp.astype(np.float64) ** 2).sum(1)[:, None]
        + xrnorm[None, :]
        - 2.0 * samp.astype(np.float64) @ x_ref.T.astype(np.float64)
    )
    # fit on the (sampled) real-data range only; pads land outside but are
    # never accumulated — op1 merely has to stay finite there (it does).
    u_lo = max(1.0, d2s.min() - 15.0)
    u_hi = d2s.max() + 15.0

    return xrp, blocks, a_spans, v_spans, n_pad, u_lo, u_hi


def kernel(x, x_ref, y, y_ref):
    x = np.asarray(x)
    x_ref = np.asarray(x_ref)
    y = np.asarray(y).astype(np.int64)
    y_ref_i = np.asarray(y_ref).astype(np.int64)

    xrp, blocks, a_spans, v_spans, n_pad, u_lo, u_hi = _prepare(
        x, x_ref, y_ref_i
    )
    ch, cq = _coeffs(u_lo, u_hi)

    key = (n_pad, tuple(a_spans), tuple(v_spans), ch, cq)
    if key not in _MODULE_CACHE:
        _MODULE_CACHE[key] = _build_module(n_pad, a_spans, v_spans, ch, cq)
    nc, names = _MODULE_CACHE[key]

    in_maps = [
        {names["wts"]: blocks[core], names["xrp"]: xrp}
        for core in range(N_CORES)
    ]

    trace = bool(int(os.environ.get("KERNEL_TRACE", "0")))
    res = run_bass_kernel_spmd(
        nc, in_maps, core_ids=list(range(N_CORES)), trace=trace
    )
    LAST["exec_time_ns"] = res.exec_time_ns
    LAST["results"] = res
    LAST["module"] = nc

    cs_parts = []
    for core in range(N_CORES):
        cl = np.asarray(res.results[core][names["cls"]], dtype=np.float64)
        part = cl[:, :C] + cl[:, C : 2 * C]
        part[:, C - 1] += cl[:, 2 * C] + cl[:, 2 * C + 1]
        cs_parts.append(part)
    cs = np.concatenate(cs_parts, axis=0)  # [B, C]

    total = cs.sum(axis=1, keepdims=True)
    soft = cs / total + 1e-6
    loss = -np.mean(np.log(soft[np.arange(B), y]))
    return np.asarray(loss, dtype=np.float32)
